# revision 15
# baseline (speedup 1.0000x reference)
"""Trainium2 Bass kernel for nn_ConvGraphSelfLoop.

out = where(any(adj>=0, axes -1,-2), relu(features @ W + b), features)

Sharding: B*V = 65536 vertices split evenly across 8 NeuronCores (8192
each); W/bias replicated; no cross-core communication.

v3 design (vs the staged fp32 baseline):
  - All wire traffic is bf16 (features/weights/output) or int8
    (adjacency sign bytes): 53MB/core instead of 109MB/core, so the
    kernel is PE-bound, not DMA-bound.
  - Features are transposed and chunk-blocked on the host, so the PE
    runs *only* the 16 N=512 bf16 matmuls per 128-token tile — no PE
    transposes and no bias matmuls (bias==0 fast path; a with-bias
    variant is compiled lazily if ever needed).
  - W rides the idle GPSIMD DMA ring chunk-by-chunk; output stores ride
    the ACT ring, so neither queues ahead of the feature loads.
  - Warmup matmuls walk the PE clock ramp during the initial DMA fill;
    the first xT group loads in per-chunk slices so real matmuls start
    ~1.2us in; the last tiles evict/store per u-half to shorten the
    drain tail.

Per core, per 128-token tile:
  - PE: per u-half: 8 accumulated bf16 matmuls
        (lhsT = xT chunk [128f,128t] stationary, rhs = W [128f,512u])
  - ACT: r = relu(psum * mask) -> bf16 (mask is per-row {0,1})
  - DVE: mask from adjacency sign bytes; xc = x*(1-mask); out = r + xc
"""
import numpy as np
import ml_dtypes
import concourse.bass as bass
import concourse.bacc as bacc
import concourse.mybir as mybir
import concourse.tile as tile
from concourse.bass_utils import run_bass_kernel_spmd

B, V, E, NN = 4, 16384, 4, 32
F, U = 1024, 1024
ENN = E * NN
NCORES = 8
T = B * V // NCORES          # 8192 tokens per core
P = 128
NT = T // P                  # 64 token tiles
C = F // P                   # 8 contraction chunks
NH = U // 512                # 2 u-halves
GK = 4                       # token tiles per xT DMA group
NG = NT // GK
SG = 8                       # token tiles per adjacency DMA group

BF16 = ml_dtypes.bfloat16

f32 = mybir.dt.float32
bf = mybir.dt.bfloat16
i8 = mybir.dt.int8
AF = mybir.ActivationFunctionType
ALU = mybir.AluOpType

XT_COLS = T * F // P         # blocked xT: [128, 65536] per core
GCOLS = C * GK * P           # 4096 cols per token-tile group


def _build(with_bias=False):
    nc = bacc.Bacc("TRN2", target_bir_lowering=False, debug=False,
                   num_devices=NCORES)
    xT_d = nc.dram_tensor("featT", [P, XT_COLS], bf, kind="ExternalInput")
    x_d = nc.dram_tensor("feat", [T, F], bf, kind="ExternalInput")
    adj_d = nc.dram_tensor("adjacency", [P, NT * ENN], i8,
                           kind="ExternalInput")
    w_d = nc.dram_tensor("weight", [P, C * U], bf, kind="ExternalInput")
    if with_bias:
        bias_d = nc.dram_tensor("bias", [1, U], bf, kind="ExternalInput")
    out_d = nc.dram_tensor("out", [T, U], bf, kind="ExternalOutput")

    with tile.TileContext(nc) as tc:
        with tc.tile_pool(name="const", bufs=1) as const, \
             tc.tile_pool(name="xtp", bufs=2) as xtp, \
             tc.tile_pool(name="xp", bufs=4) as xp, \
             tc.tile_pool(name="adp", bufs=2) as adp, \
             tc.tile_pool(name="mp", bufs=4) as mp, \
             tc.tile_pool(name="rp", bufs=3) as rp, \
             tc.tile_pool(name="op", bufs=3) as op, \
             tc.tile_pool(name="psO", bufs=3, space="PSUM") as psO:

            # ---- startup constants ----
            # W chunks ride the (otherwise idle) GPSIMD DMA ring so they
            # don't serialize ahead of the first feature loads; c-major
            # order so tile 0's c=0 matmuls unblock first.
            w_st = const.tile([P, C * U], bf)
            for c in range(C):
                cs = slice(c * U, (c + 1) * U)
                nc.gpsimd.dma_start(w_st[:, cs], w_d.ap()[:, cs])
            if with_bias:
                bias_st = const.tile([1, U], bf)
                nc.sync.dma_start(bias_st[:], bias_d.ap())
                ones_f = const.tile([1, P], f32)
                nc.vector.memset(ones_f[:], 1.0)
                ones_b = const.tile([1, P], bf)
                nc.scalar.copy(ones_b[:], ones_f[:])

            # PE warmup: walk the clock-ramp p-states during the initial
            # DMA fill so the real matmuls start at full rate.
            warm_l = const.tile([1, P], bf)
            nc.vector.memset(warm_l[:], 0.0)
            warm_r = const.tile([1, 64], bf)
            nc.vector.memset(warm_r[:], 0.0)
            with tc.tile_pool(name="psW", bufs=1, space="PSUM") as psW:
                wps = psW.tile([P, 64], f32)
                for _ in range(12):
                    nc.tensor.matmul(wps[:], warm_l[:], warm_r[:],
                                     start=True, stop=True)

            adjg = None
            xg = None
            for t in range(NT):
                if t % GK == 0:
                    g = t // GK
                    xg = xtp.tile([P, GCOLS], bf, tag="xg")
                    # group 0: per-c-chunk slices so tile 0's c=0 matmul
                    # starts after ~1KB of xT, not the full 8KB group
                    nsub = C if g == 0 else 1
                    sub = GCOLS // nsub
                    for q in range(nsub):
                        nc.sync.dma_start(
                            xg[:, q * sub:(q + 1) * sub],
                            xT_d.ap()[:, g * GCOLS + q * sub:
                                      g * GCOLS + (q + 1) * sub])
                if t % SG == 0:
                    sg = t // SG
                    adjg = adp.tile([P, SG * ENN], i8, tag="adjg")
                    nc.sync.dma_start(
                        adjg[:], adj_d.ap()[:, sg * SG * ENN:
                                            (sg + 1) * SG * ENN])
                k = t % GK
                j = t % SG
                rows = slice(t * P, (t + 1) * P)

                # ---- DMA x (token-major, for the invalid-vertex path) ----
                x_t = xp.tile([P, F], bf, tag="x")
                nc.sync.dma_start(x_t[:], x_d.ap()[rows, :])

                # ---- DVE: mask pipeline (adj bytes: 0 valid, -1 not) ----
                mx = mp.tile([P, 1], i8, tag="mx")
                nc.vector.tensor_reduce(mx[:], adjg[:, j * ENN:(j + 1) * ENN],
                                        axis=mybir.AxisListType.X, op=ALU.max)
                m_s = mp.tile([P, 1], f32, tag="m_s")
                nc.vector.tensor_scalar(m_s[:], mx[:], 0, None, ALU.is_ge)
                minv = mp.tile([P, 1], f32, tag="minv")
                nc.vector.tensor_scalar(minv[:], mx[:], 0, None, ALU.is_lt)

                # ---- ACT: mask copy (washes DVE dep into ACT stream) ----
                m_act = mp.tile([P, 1], f32, tag="m_act")
                nc.scalar.copy(m_act[:], m_s[:])

                # ---- PE: 16 bf16 matmuls; ACT: relu-evict per half ----
                po = psO.tile([P, U], f32, tag="po")
                r_t = rp.tile([P, U], bf, tag="r_t")
                xc = xp.tile([P, F], bf, tag="xc")
                nc.vector.tensor_scalar(xc[:], x_t[:], minv[:], None,
                                        ALU.mult)
                out_t = op.tile([P, U], bf, tag="out_t")
                for h in range(NH):
                    cols = slice(h * 512, (h + 1) * 512)
                    if with_bias:
                        nc.tensor.matmul(po[:, cols], ones_b[:],
                                         bias_st[:, cols],
                                         start=True, stop=False)
                    for c in range(C):
                        nc.tensor.matmul(
                            po[:, cols],
                            xg[:, c * GK * P + k * P:c * GK * P + (k + 1) * P],
                            w_st[:, c * U + h * 512:c * U + (h + 1) * 512],
                            start=(c == 0 and not with_bias),
                            stop=(c == C - 1))
                    if t >= NT - 2:
                        # final tiles: per-half evict/add/store on the idle
                        # SP ring so the store init-latencies overlap
                        nc.scalar.activation(r_t[:, cols], po[:, cols],
                                             AF.Relu, scale=m_act[:])
                        nc.vector.tensor_tensor(out=out_t[:, cols],
                                                in0=r_t[:, cols],
                                                in1=xc[:, cols], op=ALU.add)
                        nc.sync.dma_start(out_d.ap()[rows, cols],
                                          out_t[:, cols])
                if t < NT - 2:
                    # relu(psum * mask): mask==0 rows -> 0
                    nc.scalar.activation(r_t[:], po[:], AF.Relu,
                                         scale=m_act[:])
                    nc.vector.tensor_tensor(out=out_t[:], in0=r_t[:],
                                            in1=xc[:], op=ALU.add)
                    # stores ride the ACT ring: keeps loads and stores off
                    # each other's queue; aggregate HBM has the headroom
                    nc.scalar.dma_start(out_d.ap()[rows, :], out_t[:])

    nc.compile()
    return nc


_nc_cache = {}


def _get_nc(with_bias=False):
    if with_bias not in _nc_cache:
        _nc_cache[with_bias] = _build(with_bias)
    return _nc_cache[with_bias]


def _block_xT(x16):
    """[T, F] bf16 -> blocked xT [128, T*F/128].

    col = g*(C*GK*P) + c*(GK*P) + k*P + m  maps to
    x[token = (g*GK + k)*P + m, f = c*128 + Ki(partition)]
    so each token-tile group is one contiguous 8KB-line DMA and each
    matmul's stationary operand is a contiguous [128, 128] slice.
    """
    a5 = x16.reshape(NG, GK, P, C, P)             # g k m c Ki
    a5 = a5.transpose(4, 0, 3, 1, 2)              # Ki g c k m
    return np.ascontiguousarray(a5.reshape(P, XT_COLS))


def _shard_inputs(inputs):
    feats = np.asarray(inputs["features"], dtype=np.float32).reshape(B * V, F)
    x16 = feats.astype(BF16)
    adj = np.asarray(inputs["adjacency"], dtype=np.int32).reshape(B * V, ENN)
    adj8 = (adj >> 24).astype(np.int8)   # sign-preserving downcast
    w32 = np.asarray(inputs["kernel"], dtype=np.float32)
    # w[f, u] -> [Ki, c*U + u] with f = c*128 + Ki
    w16 = np.ascontiguousarray(
        w32.astype(BF16).reshape(C, P, U).transpose(1, 0, 2).reshape(P, C * U))
    bias = np.asarray(inputs["bias"], dtype=np.float32).reshape(1, U)
    with_bias = bool(np.any(bias))
    in_maps = []
    for i in range(NCORES):
        s = slice(i * T, (i + 1) * T)
        a3 = adj8[s].reshape(NT, P, ENN).transpose(1, 0, 2)  # p t e
        m = {
            "featT": _block_xT(x16[s]),
            "feat": x16[s],
            "adjacency": np.ascontiguousarray(a3.reshape(P, NT * ENN)),
            "weight": w16,
        }
        if with_bias:
            m["bias"] = bias.astype(BF16)
        in_maps.append(m)
    return in_maps


def _shard_expected(expected):
    e = expected.reshape(B * V, U)
    return [e[i * T:(i + 1) * T] for i in range(NCORES)]


def kernel(adjacency, features, kernel, bias):
    in_maps = _shard_inputs({"adjacency": adjacency, "features": features,
                             "kernel": kernel, "bias": bias})
    nc = _get_nc(with_bias="bias" in in_maps[0])
    res = run_bass_kernel_spmd(nc, in_maps, list(range(NCORES)))
    out = np.concatenate([res.results[i]["out"] for i in range(NCORES)],
                         axis=0)
    return out.reshape(B, V, U).astype(np.float32)
